# revision 8
# baseline (speedup 1.0000x reference)
"""Trainium2 Bass kernel for nn_LoRALinear1d.

Math: out[b] = (W_main + a_in[b] @ a_out[b]) @ x[b] + b_main
  with a_in[b] = reshape(W_ain @ g[b], [CIN, R]),
       a_out[b] = reshape(W_aout @ g[b], [R, COUT]).

Sharding: data-parallel over batch B=8, one batch per NeuronCore (8 cores).
All adapter math is folded on-device into an effective transposed weight
W_effT[i, o] = W_main[o, i] + (a_in @ a_out)[i, o], then a tiled
[256,256] x [256, L] matmul runs over L with the bias add fused into the
PSUM->SBUF eviction.

Memory-bound problem, so x and out travel as bf16 (host converts both
ways): 16 MB read + 16 MB write per core instead of 64 MB round trip in
fp32. The host also pre-transposes the small weights (pure marshalling)
so the device fold needs no PE transposes, and pre-permutes W_ain's
columns so both adapter rows land as free-dim slices of partition 0's
a_flat row - from there the rank-2 LoRA outer product is two K=1
accumulating matmuls with no partition shuffles at all. Total rel err
~3e-3 from the bf16 roundings, far under the 2e-2 gate.

Engine queues (each engine issues its own instruction stream in order;
each queue maps to its own DMA descriptor ring, so streams don't block
each other):
  Sync    - ONE packed weight blob + bias first (5 KB descriptors, heads
            the ring ahead of x so the fold never starves), then the even
            x chunks
  Scalar  - half the PSUM evictions (bias via activation); no DMA at all
  Vector  - other half of evictions (tensor_scalar add), small fold copies
  Tensor  - adapter matvecs, rank-2 LoRA product, all main matmuls
  GpSimd  - odd x chunks, then output stores (store triggers wait on both
            eviction engines; on a dedicated queue they can't stall anyone)
"""

from contextlib import ExitStack

import ml_dtypes
import numpy as np

import concourse.bacc as bacc
import concourse.mybir as mybir
import concourse.tile as tile
from concourse.bass_utils import run_bass_kernel_spmd

B, CIN, COUT, CINFO, R, L = 8, 256, 256, 256, 2, 32768
P = 128
LC = 2048           # L elements per SBUF tile
F32 = mybir.dt.float32
BF16 = mybir.dt.bfloat16
BF16_NP = ml_dtypes.bfloat16


def _build():
    nc = bacc.Bacc("TRN2", target_bir_lowering=False, debug=False)
    x = nc.dram_tensor("x", [CIN, L], BF16, kind="ExternalInput").ap()
    # all small weights packed per-partition into one blob so the whole set
    # rides ONE dma at the head of the sync ring with 5 KB descriptors:
    # blob[p] = [wmainT rows p,p+128 | wainT rows p,p+128 | waoutT rows
    # p,p+128 | g elems p,p+128]  (wmainT[i,o]=W_main[o,i]; wainT/waoutT
    # pre-permuted as before).  Separate queues (512 B descriptors) lost
    # the packet round-robin against the 4 KB x packets and starved the
    # fold until ~26 us, stalling the main matmul stream until 27.7 us.
    wblob = nc.dram_tensor("wblob", [P, 2562], BF16, kind="ExternalInput").ap()
    bmain = nc.dram_tensor("bmain", [COUT], F32, kind="ExternalInput").ap()
    out = nc.dram_tensor("out", [COUT, L], BF16, kind="ExternalOutput").ap()

    x_v = x.rearrange("(t p) l -> p t l", p=P)
    out_v = out.rearrange("(t p) l -> p t l", p=P)
    NCH = L // LC

    with tile.TileContext(nc) as tc, ExitStack() as ctx:
        consts = ctx.enter_context(tc.tile_pool(name="consts", bufs=1))
        # 15 x buffers + 9 output buffers: loads c0-c14 all fire immediately;
        # the one wrapping load (c15) is issued from the GpSimd queue, where
        # its wait for chunk 0's freed buffer cannot block the eviction
        # engines (a buffer-wait on the Scalar queue was stalling chunk 0's
        # activations and with them the whole PE for ~10us).  The deep output
        # pool keeps early stores - which drain at 1/3 bandwidth share while
        # the read rings are busy - from wrapping the pool and stalling the PE
        xpool = ctx.enter_context(tc.tile_pool(name="xp", bufs=15))
        opool = ctx.enter_context(tc.tile_pool(name="op", bufs=9))
        pre = ctx.enter_context(tc.tile_pool(name="pre", bufs=1))

        # the weight blob + bias lead the sync ring ahead of every x chunk:
        # ~0.66 MB lands in ~1.5 us, the fold finishes by ~12 us, and the
        # main matmul stream starts as soon as chunk 0 arrives
        blob_t = pre.tile([P, 2562], BF16, name="blob")
        nc.sync.dma_start(blob_t[:], wblob)
        b_sb = consts.tile([P, COUT // P], F32)    # bias per o-tile column
        nc.sync.dma_start(b_sb[:], bmain.rearrange("(h p) -> p h", p=P))

        # x loads alternate Sync/GpSimd: the SDMA engines round-robin rings
        # at packet granularity, so two read rings against one store ring
        # keeps reads at a 2/3 bandwidth share - the read stream finishes
        # early and late chunks never starve the PE.  Odd chunks ride the
        # GpSimd ring (FIFO-ahead of the stores), NOT the Scalar ring:
        # descriptor generation for a queued 1 MB load blocks until ring
        # space frees, and with loads on the Scalar queue chunk 0's
        # activations sat behind seven of those, stalling the PE ~8us
        xts = []
        for ci in range(NCH):
            x_t = xpool.tile([P, CIN // P, LC], BF16, name="x_t")
            eng = nc.sync if ci % 2 == 0 else nc.gpsimd
            eng.dma_start(x_t[:], x_v[:, :, ci * LC:(ci + 1) * LC])
            xts.append(x_t)

        # W_effT[i_tile][i, o] (i on partitions)
        weffT = [consts.tile([P, COUT], BF16, name=f"weffT{i}") for i in range(CIN // P)]

        with tc.tile_pool(name="prepsum", bufs=1, space="PSUM") as prepsum:
            # adapter rows: a_flat[n] = sum_c W_zT[c, n] g[c], K=c on
            # partitions; partition 0 holds the full 512-wide a_flat row
            arows = {}
            for w0, nm in ((512, "ain"), (1536, "aout")):
                a_ps = prepsum.tile([1, 512], F32, name=f"aps_{nm}", tag=f"aps_{nm}")
                for h in range(2):
                    nc.tensor.matmul(
                        a_ps[:], blob_t[:, 2560 + h:2561 + h],
                        blob_t[:, w0 + h * 512:w0 + (h + 1) * 512],
                        start=(h == 0), stop=(h == 1),
                    )
                a_row = pre.tile([1, 512], F32, name=f"arow_{nm}", tag=f"arow_{nm}")
                nc.vector.tensor_copy(a_row[:], a_ps[:])
                arows[nm] = a_row

            # W_effT = W_mainT + a_in @ a_out as two accumulating K=1 rank-1
            # updates; both r-blocks are free-dim slices of partition 0's row
            for it in range(2):
                lora_ps = prepsum.tile([P, COUT], F32, name=f"lorap{it}", tag=f"lorap{it}")
                for r in range(R):
                    nc.tensor.matmul(
                        lora_ps[:],
                        arows["ain"][:, r * 256 + it * P:r * 256 + (it + 1) * P],
                        arows["aout"][:, r * 256:(r + 1) * 256],
                        start=(r == 0), stop=(r == R - 1),
                    )
                nc.vector.tensor_add(
                    weffT[it][:], blob_t[:, it * 256:(it + 1) * 256], lora_ps[:]
                )

        # main loop over L.  Per chunk: 16 matmuls into 2-bank PSUM tiles,
        # 4 evictions (split ScalarE/VectorE) converting fp32 PSUM -> bf16,
        # one 1 MB store issued from the GpSimd queue.
        pspool = ctx.enter_context(tc.tile_pool(name="psp", bufs=4, space="PSUM"))
        EV = 1024  # eviction width: 2 PSUM banks
        for ci in range(NCH):
            xmm = xts[ci]
            o_t = opool.tile([P, COUT // P, LC], BF16, name="o_t")
            # chunk 0 accumulates k=1 first: its psum writes then wait on the
            # weffT[1] add — the last fold op — so they cannot race the fold's
            # reads of the PSUM banks this pool reuses
            ks = (1, 0) if ci == 0 else (0, 1)
            for m in range(2):
                for h in range(LC // EV):
                    ps = pspool.tile([P, EV], F32, name="ps")
                    for j, k in enumerate(ks):
                        for s in range(EV // 512):
                            nc.tensor.matmul(
                                ps[:, s * 512:(s + 1) * 512],
                                weffT[k][:, m * P:(m + 1) * P],
                                xmm[:, k, h * EV + s * 512:h * EV + (s + 1) * 512],
                                start=(j == 0), stop=(j == 1),
                            )
                    osl = o_t[:, m, h * EV:(h + 1) * EV]
                    if m == 0:
                        nc.scalar.activation(
                            osl, ps[:],
                            mybir.ActivationFunctionType.Identity,
                            bias=b_sb[:, m:m + 1],
                        )
                    else:
                        nc.vector.tensor_scalar_add(osl, ps[:], b_sb[:, m:m + 1])
            nc.gpsimd.dma_start(out_v[:, :, ci * LC:(ci + 1) * LC], o_t[:])

    nc.compile()
    return nc


_NC = None
LAST_RESULTS = None  # BassKernelResults from the most recent run


def _in_maps(x, g_out, W_main, b_main, W_ain, W_aout):
    bmain = np.ascontiguousarray(b_main, dtype=np.float32)
    wmainT = np.asarray(W_main, dtype=np.float32).T          # [CIN, COUT]
    # reorder so (W_zT @ g) lands as [r, 256] in the PE output row
    wainT = (
        np.asarray(W_ain, dtype=np.float32)
        .reshape(CIN, R, CINFO).transpose(2, 1, 0).reshape(CINFO, R * CIN)
    )
    waoutT = np.asarray(W_aout, dtype=np.float32).T          # [CINFO, R*COUT]
    # partition-major blob: row p = [wmainT rows p,p+128 | wainT rows
    # p,p+128 | waoutT rows p,p+128 | g[p], g[p+128]]
    base = np.concatenate(
        [
            wmainT.reshape(2, P, COUT).transpose(1, 0, 2).reshape(P, 2 * COUT),
            wainT.reshape(2, P, 512).transpose(1, 0, 2).reshape(P, 1024),
            waoutT.reshape(2, P, 512).transpose(1, 0, 2).reshape(P, 1024),
        ],
        axis=1,
    )
    maps = []
    for b in range(B):
        g2 = np.asarray(g_out[b, :, 0], dtype=np.float32).reshape(2, P).T
        blob = np.concatenate([base, g2], axis=1).astype(BF16_NP)
        maps.append({
            "x": np.ascontiguousarray(x[b]).astype(BF16_NP),
            "wblob": np.ascontiguousarray(blob),
            "bmain": bmain,
        })
    return maps


def kernel(x, g_out, W_main, b_main, W_ain, W_aout, trace=False):
    global _NC, LAST_RESULTS
    if _NC is None:
        _NC = _build()
    maps = _in_maps(x, g_out, W_main, b_main, W_ain, W_aout)
    LAST_RESULTS = run_bass_kernel_spmd(
        _NC, maps, core_ids=list(range(B)), trace=trace
    )
    return np.stack(
        [LAST_RESULTS.results[b]["out"].astype(np.float32) for b in range(B)], axis=0
    )



# revision 12
# speedup vs baseline: 1.0531x; 1.0531x over previous
"""Trainium2 Bass kernel for nn_LoRALinear1d.

Math: out[b] = (W_main + a_in[b] @ a_out[b]) @ x[b] + b_main
  with a_in[b] = reshape(W_ain @ g[b], [CIN, R]),
       a_out[b] = reshape(W_aout @ g[b], [R, COUT]).

Sharding: data-parallel over batch B=8, one batch per NeuronCore (8 cores).
All adapter math is folded on-device into an effective transposed weight
W_effT[i, o] = W_main[o, i] + (a_in @ a_out)[i, o], then a tiled
[256,256] x [256, L] matmul runs over L with the bias add fused into the
PSUM->SBUF eviction.

Memory-bound problem, so x and out travel as bf16 (host converts both
ways): 16 MB read + 16 MB write per core instead of 64 MB round trip in
fp32. The host also pre-transposes the small weights (pure marshalling)
so the device fold needs no PE transposes, and pre-permutes W_ain's
columns so both adapter rows land as free-dim slices of partition 0's
a_flat row - from there the rank-2 LoRA outer product is two K=1
accumulating matmuls with no partition shuffles at all. Total rel err
~3e-3 from the bf16 roundings, far under the 2e-2 gate.

Engine queues (each engine issues its own instruction stream in order;
each queue maps to its own DMA descriptor ring, so streams don't block
each other):
  Sync    - ONE packed weight blob first (5 KB descriptors, heads the ring
            ahead of x so the fold never starves), then the even x chunks
  Scalar  - bias + odd x chunks (x5+ issued lazily from the main loop so
            the HWDGE ring never fills and blocks evictions), half the
            PSUM evictions (bias via activation)
  Vector  - other half of evictions (tensor_scalar add), small fold copies
  Tensor  - adapter matvecs, rank-2 LoRA product, all main matmuls
  GpSimd  - output stores ONLY, so write bytes drain continuously instead
            of bunching behind queued reads into the end of the run
"""

from contextlib import ExitStack

import ml_dtypes
import numpy as np

import concourse.bacc as bacc
import concourse.mybir as mybir
import concourse.tile as tile
from concourse.bass_utils import run_bass_kernel_spmd

B, CIN, COUT, CINFO, R, L = 8, 256, 256, 256, 2, 32768
P = 128
LC = 2048           # L elements per SBUF tile
F32 = mybir.dt.float32
BF16 = mybir.dt.bfloat16
BF16_NP = ml_dtypes.bfloat16


def _build():
    nc = bacc.Bacc("TRN2", target_bir_lowering=False, debug=False)
    x = nc.dram_tensor("x", [CIN, L], BF16, kind="ExternalInput").ap()
    # all small weights packed per-partition into one blob so the whole set
    # rides ONE dma at the head of the sync ring with 5 KB descriptors:
    # blob[p] = [wmainT rows p,p+128 | wainT rows p,p+128 | waoutT rows
    # p,p+128 | g elems p,p+128]  (wmainT[i,o]=W_main[o,i]; wainT/waoutT
    # pre-permuted as before).  Separate queues (512 B descriptors) lost
    # the packet round-robin against the 4 KB x packets and starved the
    # fold until ~26 us, stalling the main matmul stream until 27.7 us.
    wblob = nc.dram_tensor("wblob", [P, 2562], BF16, kind="ExternalInput").ap()
    bmain = nc.dram_tensor("bmain", [COUT], F32, kind="ExternalInput").ap()
    out = nc.dram_tensor("out", [COUT, L], BF16, kind="ExternalOutput").ap()

    x_v = x.rearrange("(t p) l -> p t l", p=P)
    out_v = out.rearrange("(t p) l -> p t l", p=P)
    NCH = L // LC

    with tile.TileContext(nc) as tc, ExitStack() as ctx:
        consts = ctx.enter_context(tc.tile_pool(name="consts", bufs=1))
        # 15 x buffers + 9 output buffers: even loads fire immediately, odd
        # loads are paced by the main loop, and the wrapping buffer reuse
        # (c15 over c0) resolves long before c15's lazy dma_start issues.
        # The deep output pool keeps early stores - which drain at a 1/3
        # bandwidth share while the read rings are busy - from wrapping the
        # pool and stalling the PE
        xpool = ctx.enter_context(tc.tile_pool(name="xp", bufs=15))
        opool = ctx.enter_context(tc.tile_pool(name="op", bufs=9))
        pre = ctx.enter_context(tc.tile_pool(name="pre", bufs=1))

        # the weight blob + bias lead their rings ahead of every x chunk:
        # ~0.66 MB lands in ~1.5 us, the fold finishes by ~12 us, and the
        # main matmul stream starts as soon as chunk 0 arrives
        blob_t = pre.tile([P, 2562], BF16, name="blob")
        nc.sync.dma_start(blob_t[:], wblob)
        b_sb = consts.tile([P, COUT // P], F32)    # bias per o-tile column
        nc.scalar.dma_start(b_sb[:], bmain.rearrange("(h p) -> p h", p=P))

        # ring roles: reads ride the two HWDGE rings (Sync: even chunks,
        # Scalar: odd chunks) and stores get the GpSimd ring to themselves.
        # SDMA engines round-robin rings at packet granularity, so reads
        # hold a 2/3 bandwidth share while stores drain continuously from
        # ~16 us - queueing stores behind 8 MB of reads on one SWDGE ring
        # bunched 17 MB of writes into the throttled tail of the run and
        # wrapped the output pool, stalling the PE.  Only x1/x3 are issued
        # up front on Scalar; the rest are issued lazily from the main loop
        # (<=3 outstanding) because descriptor generation for a queued 1 MB
        # load blocks the issuing engine when the ring fills, and Scalar
        # also runs half the evictions
        xts = [xpool.tile([P, CIN // P, LC], BF16, name="x_t") for _ in range(NCH)]

        def load_x(ci):
            eng = nc.sync if ci % 2 == 0 else nc.scalar
            eng.dma_start(xts[ci][:], x_v[:, :, ci * LC:(ci + 1) * LC])

        for ci in range(0, NCH, 2):
            load_x(ci)
        load_x(1)
        load_x(3)

        # W_effT[i_tile][i, o] (i on partitions)
        weffT = [consts.tile([P, COUT], BF16, name=f"weffT{i}") for i in range(CIN // P)]

        with tc.tile_pool(name="prepsum", bufs=1, space="PSUM") as prepsum:
            # adapter rows: a_flat[n] = sum_c W_zT[c, n] g[c], K=c on
            # partitions; partition 0 holds the full 512-wide a_flat row
            arows = {}
            for w0, nm in ((512, "ain"), (1536, "aout")):
                a_ps = prepsum.tile([1, 512], F32, name=f"aps_{nm}", tag=f"aps_{nm}")
                for h in range(2):
                    nc.tensor.matmul(
                        a_ps[:], blob_t[:, 2560 + h:2561 + h],
                        blob_t[:, w0 + h * 512:w0 + (h + 1) * 512],
                        start=(h == 0), stop=(h == 1),
                    )
                a_row = pre.tile([1, 512], F32, name=f"arow_{nm}", tag=f"arow_{nm}")
                nc.vector.tensor_copy(a_row[:], a_ps[:])
                arows[nm] = a_row

            # W_effT = W_mainT + a_in @ a_out as two accumulating K=1 rank-1
            # updates; both r-blocks are free-dim slices of partition 0's row
            for it in range(2):
                lora_ps = prepsum.tile([P, COUT], F32, name=f"lorap{it}", tag=f"lorap{it}")
                for r in range(R):
                    nc.tensor.matmul(
                        lora_ps[:],
                        arows["ain"][:, r * 256 + it * P:r * 256 + (it + 1) * P],
                        arows["aout"][:, r * 256:(r + 1) * 256],
                        start=(r == 0), stop=(r == R - 1),
                    )
                nc.vector.tensor_add(
                    weffT[it][:], blob_t[:, it * 256:(it + 1) * 256], lora_ps[:]
                )

        # main loop over L.  Per chunk: 16 matmuls into 2-bank PSUM tiles,
        # 4 evictions (split ScalarE/VectorE) converting fp32 PSUM -> bf16,
        # one 1 MB store issued from the GpSimd queue.
        pspool = ctx.enter_context(tc.tile_pool(name="psp", bufs=4, space="PSUM"))
        EV = 1024  # eviction width: 2 PSUM banks
        for ci in range(NCH):
            xmm = xts[ci]
            o_t = opool.tile([P, COUT // P, LC], BF16, name="o_t")
            # chunk 0 accumulates k=1 first: its psum writes then wait on the
            # weffT[1] add — the last fold op — so they cannot race the fold's
            # reads of the PSUM banks this pool reuses
            ks = (1, 0) if ci == 0 else (0, 1)
            for m in range(2):
                for h in range(LC // EV):
                    ps = pspool.tile([P, EV], F32, name="ps")
                    for j, k in enumerate(ks):
                        for s in range(EV // 512):
                            nc.tensor.matmul(
                                ps[:, s * 512:(s + 1) * 512],
                                weffT[k][:, m * P:(m + 1) * P],
                                xmm[:, k, h * EV + s * 512:h * EV + (s + 1) * 512],
                                start=(j == 0), stop=(j == 1),
                            )
                    osl = o_t[:, m, h * EV:(h + 1) * EV]
                    if m == 0:
                        nc.scalar.activation(
                            osl, ps[:],
                            mybir.ActivationFunctionType.Identity,
                            bias=b_sb[:, m:m + 1],
                        )
                    else:
                        nc.vector.tensor_scalar_add(osl, ps[:], b_sb[:, m:m + 1])
                if m == 0 and 2 * ci + 5 < NCH:
                    load_x(2 * ci + 5)
            nc.gpsimd.dma_start(out_v[:, :, ci * LC:(ci + 1) * LC], o_t[:])

    nc.compile()
    return nc


_NC = None
LAST_RESULTS = None  # BassKernelResults from the most recent run


def _in_maps(x, g_out, W_main, b_main, W_ain, W_aout):
    bmain = np.ascontiguousarray(b_main, dtype=np.float32)
    wmainT = np.asarray(W_main, dtype=np.float32).T          # [CIN, COUT]
    # reorder so (W_zT @ g) lands as [r, 256] in the PE output row
    wainT = (
        np.asarray(W_ain, dtype=np.float32)
        .reshape(CIN, R, CINFO).transpose(2, 1, 0).reshape(CINFO, R * CIN)
    )
    waoutT = np.asarray(W_aout, dtype=np.float32).T          # [CINFO, R*COUT]
    # partition-major blob: row p = [wmainT rows p,p+128 | wainT rows
    # p,p+128 | waoutT rows p,p+128 | g[p], g[p+128]]
    base = np.concatenate(
        [
            wmainT.reshape(2, P, COUT).transpose(1, 0, 2).reshape(P, 2 * COUT),
            wainT.reshape(2, P, 512).transpose(1, 0, 2).reshape(P, 1024),
            waoutT.reshape(2, P, 512).transpose(1, 0, 2).reshape(P, 1024),
        ],
        axis=1,
    )
    maps = []
    for b in range(B):
        g2 = np.asarray(g_out[b, :, 0], dtype=np.float32).reshape(2, P).T
        blob = np.concatenate([base, g2], axis=1).astype(BF16_NP)
        maps.append({
            "x": np.ascontiguousarray(x[b]).astype(BF16_NP),
            "wblob": np.ascontiguousarray(blob),
            "bmain": bmain,
        })
    return maps


def kernel(x, g_out, W_main, b_main, W_ain, W_aout, trace=False):
    global _NC, LAST_RESULTS
    if _NC is None:
        _NC = _build()
    maps = _in_maps(x, g_out, W_main, b_main, W_ain, W_aout)
    LAST_RESULTS = run_bass_kernel_spmd(
        _NC, maps, core_ids=list(range(B)), trace=trace
    )
    return np.stack(
        [LAST_RESULTS.results[b]["out"].astype(np.float32) for b in range(B)], axis=0
    )



# revision 17
# speedup vs baseline: 1.0690x; 1.0150x over previous
"""Trainium2 Bass kernel for nn_LoRALinear1d.

Math: out[b] = (W_main + a_in[b] @ a_out[b]) @ x[b] + b_main
  with a_in[b] = reshape(W_ain @ g[b], [CIN, R]),
       a_out[b] = reshape(W_aout @ g[b], [R, COUT]).

Sharding: data-parallel over batch B=8, one batch per NeuronCore (8 cores).
All adapter math is folded on-device into an effective transposed weight
W_effT[i, o] = W_main[o, i] + (a_in @ a_out)[i, o], then a tiled
[256,256] x [256, L] matmul runs over L with the bias add fused into the
PSUM->SBUF eviction.

Memory-bound problem, so x and out travel as bf16 (host converts both
ways): 16 MB read + 16 MB write per core instead of 64 MB round trip in
fp32. The host also pre-transposes the small weights (pure marshalling)
so the device fold needs no PE transposes, and pre-permutes W_ain's
columns so both adapter rows land as free-dim slices of partition 0's
a_flat row - from there the rank-2 LoRA outer product is two K=1
accumulating matmuls with no partition shuffles at all. Total rel err
~3e-3 from the bf16 roundings, far under the 2e-2 gate.

Engine queues (each engine issues its own instruction stream in order;
each queue maps to its own DMA descriptor ring, so streams don't block
each other):
  Sync    - ONE packed weight blob first (5 KB descriptors, heads the ring
            ahead of x so the fold never starves), then ALL 16 x chunks,
            paced to <=8 in flight by the xpool buffer-reuse semaphores
  Scalar  - half the PSUM evictions (bias via activation); no DMA
  Vector  - other half of evictions (tensor_scalar add), small fold copies
  Tensor  - adapter matvecs, rank-2 LoRA product, all main matmuls
  GpSimd  - bias, then ALL output stores, gated behind the delivery of the
            last x chunk so reads and writes never interleave on the HBM
"""

from contextlib import ExitStack

import ml_dtypes
import numpy as np

import concourse.bacc as bacc
import concourse.mybir as mybir
import concourse.tile as tile
from concourse.bass_utils import run_bass_kernel_spmd

B, CIN, COUT, CINFO, R, L = 8, 256, 256, 256, 2, 32768
P = 128
LC = 2048           # L elements per SBUF tile
F32 = mybir.dt.float32
BF16 = mybir.dt.bfloat16
BF16_NP = ml_dtypes.bfloat16


def _build():
    nc = bacc.Bacc("TRN2", target_bir_lowering=False, debug=False)
    x = nc.dram_tensor("x", [CIN, L], BF16, kind="ExternalInput").ap()
    # all small weights packed per-partition into one blob so the whole set
    # rides ONE dma at the head of the sync ring with 5 KB descriptors:
    # blob[p] = [wmainT rows p,p+128 | wainT rows p,p+128 | waoutT rows
    # p,p+128 | g elems p,p+128]  (wmainT[i,o]=W_main[o,i]; wainT/waoutT
    # pre-permuted as before).  Separate queues (512 B descriptors) lost
    # the packet round-robin against the 4 KB x packets and starved the
    # fold until ~26 us, stalling the main matmul stream until 27.7 us.
    wblob = nc.dram_tensor("wblob", [P, 2562], BF16, kind="ExternalInput").ap()
    bmain = nc.dram_tensor("bmain", [COUT], F32, kind="ExternalInput").ap()
    out = nc.dram_tensor("out", [COUT, L], BF16, kind="ExternalOutput").ap()

    x_v = x.rearrange("(t p) l -> p t l", p=P)
    out_v = out.rearrange("(t p) l -> p t l", p=P)
    NCH = L // LC

    with tile.TileContext(nc) as tc, ExitStack() as ctx:
        consts = ctx.enter_context(tc.tile_pool(name="consts", bufs=1))
        # xpool bufs=8 doubles as the load pacer: x_j's dma_start waits for
        # the PE to finish x_{j-8}, keeping <=8 loads in flight, which both
        # respects the 8 DMAHW completion lanes (a 9th concurrent dma_start
        # stalls its engine until a lane frees) and keeps issue order =
        # consumption order.  opool bufs=12 lets every chunk evict without
        # ever waiting on the held-back stores (store0 completes ~54us,
        # first wrap need at chunk 12 ~62us)
        xpool = ctx.enter_context(tc.tile_pool(name="xp", bufs=8))
        opool = ctx.enter_context(tc.tile_pool(name="op", bufs=12))
        pre = ctx.enter_context(tc.tile_pool(name="pre", bufs=1))

        # the weight blob leads the read ring ahead of every x chunk:
        # ~0.66 MB lands in ~1.5 us, the fold finishes by ~12 us, and the
        # main matmul stream starts as soon as chunk 0 arrives
        blob_t = pre.tile([P, 2562], BF16, name="blob")
        nc.sync.dma_start(blob_t[:], wblob)
        b_sb = consts.tile([P, COUT // P], F32)    # bias per o-tile column
        nc.gpsimd.dma_start(b_sb[:], bmain.rearrange("(h p) -> p h", p=P))

        # phase-separated DMA: ALL reads ride the sync ring, ALL writes the
        # gpsimd ring, and the writes are gated (see the release copy below)
        # until the last read is in SBUF.  When only one ring has work all
        # 16 SDMA engines serve it at line rate, so each phase runs at the
        # full ~420 GB/s; overlapping read and write streams measurably
        # degrades per-engine throughput ~15% (HBM read/write turnarounds),
        # which is what made mixed-phase schedules lose to this serial one
        xts = [xpool.tile([P, CIN // P, LC], BF16, name="x_t") for _ in range(NCH)]
        for ci in range(NCH):
            nc.sync.dma_start(xts[ci][:], x_v[:, :, ci * LC:(ci + 1) * LC])

        # W_effT[i_tile][i, o] (i on partitions)
        weffT = [consts.tile([P, COUT], BF16, name=f"weffT{i}") for i in range(CIN // P)]

        with tc.tile_pool(name="prepsum", bufs=1, space="PSUM") as prepsum:
            # adapter rows: a_flat[n] = sum_c W_zT[c, n] g[c], K=c on
            # partitions; partition 0 holds the full 512-wide a_flat row
            arows = {}
            for w0, nm in ((512, "ain"), (1536, "aout")):
                a_ps = prepsum.tile([1, 512], F32, name=f"aps_{nm}", tag=f"aps_{nm}")
                for h in range(2):
                    nc.tensor.matmul(
                        a_ps[:], blob_t[:, 2560 + h:2561 + h],
                        blob_t[:, w0 + h * 512:w0 + (h + 1) * 512],
                        start=(h == 0), stop=(h == 1),
                    )
                a_row = pre.tile([1, 512], F32, name=f"arow_{nm}", tag=f"arow_{nm}")
                nc.vector.tensor_copy(a_row[:], a_ps[:])
                arows[nm] = a_row

            # W_effT = W_mainT + a_in @ a_out as two accumulating K=1 rank-1
            # updates; both r-blocks are free-dim slices of partition 0's row
            for it in range(2):
                lora_ps = prepsum.tile([P, COUT], F32, name=f"lorap{it}", tag=f"lorap{it}")
                for r in range(R):
                    nc.tensor.matmul(
                        lora_ps[:],
                        arows["ain"][:, r * 256 + it * P:r * 256 + (it + 1) * P],
                        arows["aout"][:, r * 256:(r + 1) * 256],
                        start=(r == 0), stop=(r == R - 1),
                    )
                nc.vector.tensor_add(
                    weffT[it][:], blob_t[:, it * 256:(it + 1) * 256], lora_ps[:]
                )

        # store release gate: a 1-element GpSimd read of the last x tile.
        # The in-order GpSimd queue then holds every store dma_start behind
        # the delivery of x15, keeping the write phase off the HBM until
        # the read phase is done
        gate = pre.tile([1, 1], BF16, name="gate")
        nc.gpsimd.tensor_copy(gate[:], xts[NCH - 1][0:1, 0, 0:1])

        # main loop over L.  Per chunk: 16 matmuls into 2-bank PSUM tiles,
        # 4 evictions (split ScalarE/VectorE) converting fp32 PSUM -> bf16,
        # one 1 MB store issued from the GpSimd queue.
        pspool = ctx.enter_context(tc.tile_pool(name="psp", bufs=4, space="PSUM"))
        EV = 1024  # eviction width: 2 PSUM banks
        for ci in range(NCH):
            xmm = xts[ci]
            o_t = opool.tile([P, COUT // P, LC], BF16, name="o_t")
            # chunk 0 accumulates k=1 first: its psum writes then wait on the
            # weffT[1] add — the last fold op — so they cannot race the fold's
            # reads of the PSUM banks this pool reuses
            ks = (1, 0) if ci == 0 else (0, 1)
            for m in range(2):
                for h in range(LC // EV):
                    ps = pspool.tile([P, EV], F32, name="ps")
                    for j, k in enumerate(ks):
                        for s in range(EV // 512):
                            nc.tensor.matmul(
                                ps[:, s * 512:(s + 1) * 512],
                                weffT[k][:, m * P:(m + 1) * P],
                                xmm[:, k, h * EV + s * 512:h * EV + (s + 1) * 512],
                                start=(j == 0), stop=(j == 1),
                            )
                    osl = o_t[:, m, h * EV:(h + 1) * EV]
                    if m == 0:
                        nc.scalar.activation(
                            osl, ps[:],
                            mybir.ActivationFunctionType.Identity,
                            bias=b_sb[:, m:m + 1],
                        )
                    else:
                        nc.vector.tensor_scalar_add(osl, ps[:], b_sb[:, m:m + 1])
            nc.gpsimd.dma_start(out_v[:, :, ci * LC:(ci + 1) * LC], o_t[:])

    nc.compile()
    return nc


_NC = None
LAST_RESULTS = None  # BassKernelResults from the most recent run


def _in_maps(x, g_out, W_main, b_main, W_ain, W_aout):
    bmain = np.ascontiguousarray(b_main, dtype=np.float32)
    wmainT = np.asarray(W_main, dtype=np.float32).T          # [CIN, COUT]
    # reorder so (W_zT @ g) lands as [r, 256] in the PE output row
    wainT = (
        np.asarray(W_ain, dtype=np.float32)
        .reshape(CIN, R, CINFO).transpose(2, 1, 0).reshape(CINFO, R * CIN)
    )
    waoutT = np.asarray(W_aout, dtype=np.float32).T          # [CINFO, R*COUT]
    # partition-major blob: row p = [wmainT rows p,p+128 | wainT rows
    # p,p+128 | waoutT rows p,p+128 | g[p], g[p+128]]
    base = np.concatenate(
        [
            wmainT.reshape(2, P, COUT).transpose(1, 0, 2).reshape(P, 2 * COUT),
            wainT.reshape(2, P, 512).transpose(1, 0, 2).reshape(P, 1024),
            waoutT.reshape(2, P, 512).transpose(1, 0, 2).reshape(P, 1024),
        ],
        axis=1,
    )
    maps = []
    for b in range(B):
        g2 = np.asarray(g_out[b, :, 0], dtype=np.float32).reshape(2, P).T
        blob = np.concatenate([base, g2], axis=1).astype(BF16_NP)
        maps.append({
            "x": np.ascontiguousarray(x[b]).astype(BF16_NP),
            "wblob": np.ascontiguousarray(blob),
            "bmain": bmain,
        })
    return maps


def kernel(x, g_out, W_main, b_main, W_ain, W_aout, trace=False):
    global _NC, LAST_RESULTS
    if _NC is None:
        _NC = _build()
    maps = _in_maps(x, g_out, W_main, b_main, W_ain, W_aout)
    LAST_RESULTS = run_bass_kernel_spmd(
        _NC, maps, core_ids=list(range(B)), trace=trace
    )
    return np.stack(
        [LAST_RESULTS.results[b]["out"].astype(np.float32) for b in range(B)], axis=0
    )

